# revision 2
# baseline (speedup 1.0000x reference)
"""MHA kernel for TRN2 v4: x[8,512,32,32], 8 heads, S=1024, C=512.

Sharding: data-parallel over batch N=8 -> one batch item per NeuronCore.
Per-core layout (all transpose-free):
  qkT[e,s]  = w.T @ x                     (e on partitions)
  v[s,e]    = x.T @ w_v                   (s on partitions)
  scoresT   = kT_h.T @ qT_h               (k_s on partitions; K=64 row-tiled pair)
  P         = exp(scoresT/8)              (split: ACT exact / DVE Schraudolph-int16)
  oT_aug    = [v_h | 1].T @ P             (M=65; row 64 = softmax denominator r)
  oT        = oT_aug[:64] * (1/r)         (DVE recip + gpsimd broadcast + DVE mul)
  yT[o,s]   = w_outT.T @ oT               (bf16 out; bias host-side)

v4: host pre-packs x/w/wo into chunk-major [128, *] layouts so all input DMA is
5 large contiguous transfers on one queue in need order (x; q0k0q1k1; v;
q2k2q3k3; wo). PE warmup MMs bridge the DMA window (DCE-proofed via wsink
output). QK-before-PV lag, Schraudolph exp on DVE 4 tiles/step, step-4
group-major PV with staggered normalize and overlapped out-proj waves, y DMAs
interleaved per-chunk on alternating queues.
"""

import numpy as np
import ml_dtypes

import concourse.bacc as bacc
import concourse.mybir as mybir
import concourse.tile as tile
from concourse.bass_utils import run_bass_kernel_spmd

P = 128
S = 1024          # sequence = 32*32
C = 512           # channels
NH = 8            # heads
HD = 64           # head dim
CT = C // P       # 4 c-tiles
MT = S // P       # 8 s-tiles
BF = mybir.dt.bfloat16
F32 = mybir.dt.float32
I16 = mybir.dt.int16
LAG = 3           # PV trails QK by LAG mt-slots within a step

# Schraudolph exp: bf16 bits = round(A*s + B), folding the 1/sqrt(64) scale
SCH_A = float((2.0**7) / np.log(2.0) / 8.0)
SCH_B = float(127 * 2**7 - 7.5)

# which (mt, nt) slots per step use DVE Schraudolph instead of ACT exp
DVE_SLOTS = {0: (), 1: (1, 5, 9, 13), 2: (1, 5, 9, 13), 3: (1, 5, 9, 13), 4: ()}

_cache = {}


def build_program():
    nc = bacc.Bacc("TRN2", target_bir_lowering=False, debug=False, num_devices=8)
    # host pre-packed layouts (see _prep)
    x_d = nc.dram_tensor("x", [P, CT * S], BF, kind="ExternalInput").ap()
    wq_d = nc.dram_tensor("wq", [P, CT * 3 * C], BF, kind="ExternalInput").ap()
    wo_d = nc.dram_tensor("wo", [P, CT * C], BF, kind="ExternalInput").ap()
    y_d = nc.dram_tensor("y", [C, S], BF, kind="ExternalOutput").ap()
    wsink_d = nc.dram_tensor("wsink", [1, 8], BF, kind="ExternalOutput").ap()

    with tile.TileContext(nc) as tc:
        with (
            tc.tile_pool(name="const", bufs=1) as cpool,
            tc.tile_pool(name="qk", bufs=1) as qkpool,
            tc.tile_pool(name="vp", bufs=1) as vpool,
            tc.tile_pool(name="pp", bufs=32) as ppool,
            tc.tile_pool(name="ot", bufs=1) as opool,
            tc.tile_pool(name="misc", bufs=4) as mpool,
            tc.tile_pool(name="psq", bufs=3, space="PSUM") as psq_pool,
            tc.tile_pool(name="pso", bufs=2, space="PSUM") as pso_pool,
        ):
            # ---- PE warmup: keep HAM busy while input DMAs land ----
            warm = cpool.tile([P, 512], BF, name="warm", tag="warm")
            nc.scalar.memzero(warm[:])
            warm_ps = psq_pool.tile([P, 1024], F32, name="wps", tag="psq")
            for _ in range(14):
                nc.tensor.matmul(
                    warm_ps[:, 0:512], warm[:, 0:128], warm[:],
                    start=True, stop=True,
                )
            wsink = cpool.tile([1, 8], BF, name="wsink", tag="wsink")
            nc.vector.tensor_copy(wsink[:], warm_ps[0:1, 0:8])

            # ---- load inputs: 7 contiguous DMAs, one queue, need order ----
            # x host layout: [p, nt*2048 + ct*512 + s']  (nt-major)
            x_sb = cpool.tile([P, CT * S], BF, name="xall", tag="xall")
            w_sb = cpool.tile([P, CT * 3 * C], BF, name="wall", tag="wall")
            wo_sb = cpool.tile([P, CT * C], BF, name="woall", tag="woall")
            # w host layout: [q0k0 | q1k1 | v | q2k2 | q3k3], ct-major inside
            nc.sync.dma_start(w_sb[:, 0:1024], wq_d[:, 0:1024])        # q0k0
            nc.sync.dma_start(x_sb[:, 0:2048], x_d[:, 0:2048])         # x nt0
            nc.sync.dma_start(x_sb[:, 2048:4096], x_d[:, 2048:4096])   # x nt1
            nc.sync.dma_start(w_sb[:, 1024:2048], wq_d[:, 1024:2048])  # q1k1
            nc.sync.dma_start(w_sb[:, 2048:4096], wq_d[:, 2048:4096])  # v
            nc.sync.dma_start(w_sb[:, 4096:6144], wq_d[:, 4096:6144])  # q2k2 q3k3
            nc.sync.dma_start(wo_sb[:], wo_d[:, :])
            nc.sync.dma_start(wsink_d[:, :], wsink[:])
            W_PAIR_BASE = (0, 1024, 4096, 5120)

            def xs(ct, lo, hi):
                # x cols [lo:hi) of c-tile ct in nt-major layout (hi-lo <= 512
                # and the range must not straddle the nt boundary at 512)
                nt, off = lo // 512, lo % 512
                base = nt * 2048 + ct * 512 + off
                return x_sb[:, base: base + (hi - lo)]

            # ---- qkT projection: qk_sb[pair] = q-pair, qk_sb[4+pair] = k-pair ----
            qk_sb = [qkpool.tile([P, S], BF, name=f"qk{et}", tag=f"qk{et}")
                     for et in range(8)]
            v_sb = [None] * MT

            def emit_qk_group(pair, kq, nt):
                ps = pso_pool.tile([P, 512], F32, name="qp", tag="pso")
                for ct in range(CT):
                    base = W_PAIR_BASE[pair] + ct * 256 + kq * 128
                    nc.tensor.matmul(
                        ps[:],
                        w_sb[:, base:base + 128],
                        xs(ct, nt * 512, (nt + 1) * 512),
                        start=(ct == 0), stop=(ct == CT - 1),
                    )
                nc.vector.tensor_copy(
                    qk_sb[4 * kq + pair][:, nt * 512:(nt + 1) * 512], ps[:]
                )

            def emit_v_group(mt):
                ps = pso_pool.tile([P, 512], F32, name="vps", tag="pso")
                for ct in range(CT):
                    nc.tensor.matmul(
                        ps[:],
                        xs(ct, mt * P, (mt + 1) * P),
                        w_sb[:, 2048 + ct * 512: 2048 + (ct + 1) * 512],
                        start=(ct == 0), stop=(ct == CT - 1),
                    )
                vt = vpool.tile([P, NH * (HD + 1)], BF, name=f"v{mt}", tag=f"v{mt}")
                nc.gpsimd.memset(vt[:], 1.0)
                dst = vt[:].rearrange("p (h e) -> p h e", e=HD + 1)[:, :, 0:HD]
                nc.vector.tensor_copy(dst, ps[:].rearrange("p (h e) -> p h e", e=HD))
                v_sb[mt] = vt

            # block A: what pair-0 attention needs first
            for pair, kq, nt in ((0, 0, 0), (0, 1, 0), (0, 0, 1), (0, 1, 1)):
                emit_qk_group(pair, kq, nt)
            pending = [("qk", 1, 0, 0), ("qk", 1, 1, 0), ("qk", 1, 0, 1), ("qk", 1, 1, 1)]
            pending += [("v", mt, None, None) for mt in range(MT)]
            pending += [("qk", pr, kq, nt) for pr in (2, 3) for kq in (0, 1)
                        for nt in (0, 1)]
            pend_i = 0

            # ---- attention, software-pipelined: QK/exp(pair p) || PV(pair p-1) ----
            oT_sb = [opool.tile([P, S], BF, name=f"o{ct}", tag=f"o{ct}")
                     for ct in range(CT)]
            y_sb = [opool.tile([P, 512], BF, name=f"ysb{i}", tag=f"ysb{i}")
                    for i in range(8)]
            p_tiles = {}
            DRAIN_ORDER = ((0, 0), (1, 0), (0, 1), (1, 1))

            def emit_qk_slot(step, mt, nt, dve_tiles=()):
                psq = psq_pool.tile([P, 1024], F32, name="psq", tag="psq")
                for hh in range(2):
                    nc.tensor.matmul(
                        psq[:, hh * 512:(hh + 1) * 512],
                        qk_sb[4 + step][hh * HD:(hh + 1) * HD, mt * P:(mt + 1) * P],
                        qk_sb[step][hh * HD:(hh + 1) * HD, nt * 512:(nt + 1) * 512],
                        start=True, stop=True,
                    )
                pt = ppool.tile([P, 1024], BF, name="ptile", tag="ptile")
                if mt * 2 + nt in DVE_SLOTS[step]:
                    nc.vector.tensor_scalar(
                        pt[:].bitcast(I16), psq[:], SCH_A, SCH_B,
                        mybir.AluOpType.mult, mybir.AluOpType.add,
                    )
                else:
                    nc.scalar.activation(
                        pt[:], psq[:], mybir.ActivationFunctionType.Exp,
                        scale=float(1.0 / np.sqrt(HD)),
                    )
                p_tiles[(step, mt, nt)] = pt

            def emit_pv(pp_, mt, idx, hh, nt, pso_t):
                h = 2 * pp_ + hh
                nc.tensor.matmul(
                    pso_t[idx][0:HD + 1, :],
                    v_sb[mt][:, h * (HD + 1):(h + 1) * (HD + 1)],
                    p_tiles[(pp_, mt, nt)][:, hh * 512:(hh + 1) * 512],
                    start=(mt == 0), stop=(mt == MT - 1),
                )

            def normalize(pp_, idx, hh, nt, pso_t, copy_eng="v"):
                h = 2 * pp_ + hh
                ct, half = h // 2, h % 2
                rrow = mpool.tile([1, 512], F32, name="rrow", tag="rrow")
                if copy_eng == "v":
                    nc.vector.tensor_copy(rrow[0:1, :], pso_t[idx][HD:HD + 1, :])
                else:
                    nc.scalar.copy(rrow[0:1, :], pso_t[idx][HD:HD + 1, :])
                rinv = mpool.tile([1, 512], F32, name="rinv", tag="rinv")
                nc.vector.reciprocal_approx_fast(rinv[0:1, :], rrow[0:1, :])
                bc = mpool.tile([HD, 512], F32, name="bc", tag="bc")
                nc.gpsimd.partition_broadcast(bc[:], rinv[0:1, :], channels=HD)
                nc.vector.tensor_mul(
                    oT_sb[ct][half * HD:(half + 1) * HD, nt * 512:(nt + 1) * 512],
                    pso_t[idx][0:HD, :], bc[:],
                )

            def proj_mm(ot, ct, st_, chunk, start, stop):
                nc.tensor.matmul(
                    chunk,
                    wo_sb[:, ct * 512 + ot * P: ct * 512 + (ot + 1) * P],
                    oT_sb[ct][:, st_ * 512:(st_ + 1) * 512],
                    start=start, stop=stop, skip_group_check=True,
                )

            def emit_y_one(ot, st_, chunk, copy_eng, dma_eng):
                dst = y_sb[st_ * 4 + ot][:]
                if copy_eng == "v":
                    nc.vector.tensor_copy(dst, chunk)
                else:
                    nc.scalar.copy(dst, chunk)
                eng = nc.gpsimd if dma_eng == "g" else nc.sync
                eng.dma_start(
                    y_d[ot * P:(ot + 1) * P, st_ * 512:(st_ + 1) * 512], dst)

            # step 0: QK/exp for pair 0 + paced drain of remaining qkv groups
            for mt in range(MT):
                for nt in range(2):
                    emit_qk_slot(0, mt, nt)
                    slot = mt * 2 + nt
                    want = 20 * (slot + 1) // 16
                    while pend_i < min(want, 20):
                        kind, i1, i2, i3 = pending[pend_i]
                        if kind == "qk":
                            emit_qk_group(i1, i2, i3)
                        else:
                            emit_v_group(i1)
                        pend_i += 1

            # steps 1..3: QK slots interleaved with group-major PV of pair-1;
            # each PV group normalizes mid-step, freeing its PSUM bank early
            for step in range(1, 4):
                cur = [None] * 4
                for j in range(16):
                    emit_qk_slot(step, j // 2, j % 2)
                    for k in (2 * j, 2 * j + 1):
                        g, mt = k // MT, k % MT
                        hh, nt = DRAIN_ORDER[g]
                        if mt == 0:
                            cur[g] = pso_pool.tile([P, 512], F32,
                                                   name=f"pso{g}", tag="pso")
                        emit_pv(step - 1, mt, g, hh, nt, cur)
                        if mt == MT - 1:
                            normalize(step - 1, g, hh, nt, cur)

            # step 4: PV pair 3 group-major + staggered normalize + out-proj waves
            pso_t4 = [None] * 4
            wvA = [psq_pool.tile([P, 1024], F32, name=f"wa{i}", tag="psq")
                   for i in range(2)]
            wvA_c = [wvA[0][:, 0:512], wvA[0][:, 512:1024],
                     wvA[1][:, 0:512], wvA[1][:, 512:1024]]
            for g, (hh, nt) in enumerate(DRAIN_ORDER):
                pso_t4[g] = pso_pool.tile([P, 512], F32, name=f"ps4{g}", tag="pso")
                for mt in range(MT):
                    emit_pv(3, mt, g, hh, nt, pso_t4)
                normalize(3, g, hh, nt, pso_t4,
                          copy_eng="s" if g < 3 else "v")
                if g == 1:
                    for ct in (0, 1, 2):
                        for ot in range(CT):
                            proj_mm(ot, ct, 0, wvA_c[ot], ct == 0, False)
                if g == 2:
                    for ot in range(CT):
                        proj_mm(ot, 3, 0, wvA_c[ot], False, True)
                        emit_y_one(ot, 0, wvA_c[ot], "s", "s")
            # wave B: on recycled psq tiles
            wvB = [psq_pool.tile([P, 1024], F32, name=f"wb{i}", tag="psq")
                   for i in range(2)]
            wvB_c = [wvB[0][:, 0:512], wvB[0][:, 512:1024],
                     wvB[1][:, 0:512], wvB[1][:, 512:1024]]
            for ct in (0, 1, 2):
                for ot in range(CT):
                    proj_mm(ot, ct, 1, wvB_c[ot], ct == 0, False)
            for ot in range(CT):
                proj_mm(ot, 3, 1, wvB_c[ot], False, True)
                emit_y_one(ot, 1, wvB_c[ot],
                           "v" if ot < 2 else "s", "s")

    nc.compile()
    return nc


def get_program():
    if "nc" not in _cache:
        _cache["nc"] = build_program()
    return _cache["nc"]


def _prep(x, w_qkv, w_out):
    N = x.shape[0]
    # x: [N, C, H, W] -> [N, 128, nt*2048 + ct*512 + s'] (nt-major)
    xb = (x.reshape(N, CT, P, 2, 512).transpose(0, 2, 3, 1, 4)
          .reshape(N, P, CT * S))
    xb = np.ascontiguousarray(xb).astype(ml_dtypes.bfloat16)
    # w: [3C, C] -> wqT [C, 3C] -> [q0k0 | q1k1 | v | q2k2 | q3k3], ct-major
    wqT = w_qkv.T.astype(np.float32)                    # [C, 3C] cols e
    W = wqT.reshape(CT, P, 3 * C)                       # [ct, p, e]

    def ctmaj(b):                                       # [ct, p, w] -> [p, ct*w]
        return b.transpose(1, 0, 2).reshape(P, -1)

    pairs = [
        np.concatenate([W[:, :, pr * 128:(pr + 1) * 128],
                        W[:, :, 512 + pr * 128:512 + (pr + 1) * 128]], axis=2)
        for pr in range(4)
    ]
    vblk = W[:, :, 1024:1536]
    wq2 = np.concatenate(
        [ctmaj(pairs[0]), ctmaj(pairs[1]), ctmaj(vblk),
         ctmaj(pairs[2]), ctmaj(pairs[3])], axis=1)
    wq2 = np.ascontiguousarray(wq2).astype(ml_dtypes.bfloat16)
    woT = w_out.T.astype(np.float32)                    # [C, C]
    wo2 = woT.reshape(CT, P, C).transpose(1, 0, 2).reshape(P, CT * C)
    wo2 = np.ascontiguousarray(wo2).astype(ml_dtypes.bfloat16)
    return xb, wq2, wo2


def kernel(x, w_qkv, w_out, b_out, _trace=False, _tmpdir=None):
    x = np.asarray(x, dtype=np.float32)
    w_qkv = np.asarray(w_qkv, dtype=np.float32)
    w_out = np.asarray(w_out, dtype=np.float32)
    b_out = np.asarray(b_out, dtype=np.float32)
    N = x.shape[0]

    xb, wq2, wo2 = _prep(x, w_qkv, w_out)
    nc = get_program()
    in_maps = [
        {"x": np.ascontiguousarray(xb[n]), "wq": wq2, "wo": wo2}
        for n in range(N)
    ]
    res = run_bass_kernel_spmd(
        nc, in_maps, core_ids=list(range(N)), trace=_trace, tmpdir=_tmpdir
    )
    y = np.stack([np.asarray(res.results[n]["y"]).astype(np.float32)
                  for n in range(N)])
    y = y.reshape(N, C, 32, 32)
    y = y + b_out[None, :, None, None]
    if _trace:
        return y, res
    return y


# revision 3
# speedup vs baseline: 1.0110x; 1.0110x over previous
"""MHA kernel for TRN2: x[8,512,32,32], 8 heads, S=1024, C=512.

Sharding: data-parallel over batch N=8 -> one batch item per NeuronCore.
Per-core layout (all transpose-free):
  qkT[e,s]  = w.T @ x                     (e on partitions)
  v[s,e]    = x.T @ w_v                   (s on partitions)
  scoresT   = kT_h.T @ qT_h               (k_s on partitions; K=64 row-tiled head
                                           pair runs concurrently on the PE)
  P         = exp(scoresT/8)              (ACT exact; 4 tiles/step on DVE via a
                                           Schraudolph int16 bit-trick)
  oT_aug    = [v_h | 1].T @ P             (M=65; row 64 = softmax denominator r)
  oT        = oT_aug[:64] * (1/r)         (DVE recip + gpsimd broadcast + DVE mul)
  yT[o,s]   = w_outT.T @ oT               (bf16 out; bias added host-side)

Schedule: host pre-packs x/w/wo into [128, *] layouts so input DMA is 7 large
contiguous need-ordered transfers on one HWDGE queue. 14 warmup matmuls bridge
the DMA window so the PE HAM clock is warm when real work starts (kept alive by
the wsink output). Attention is software-pipelined: QK/exp of pair p overlaps
PV of pair p-1; PV runs group-major so each softmax normalize happens mid-step
and frees its PSUM bank early. PSUM: 3x[128,1024] score tiles (loose QK<->exp
coupling) + 2 rotating PV banks. The output projection's two 512-column waves
overlap the final PV drain on recycled PSUM banks.
"""

import numpy as np
import ml_dtypes

import concourse.bacc as bacc
import concourse.mybir as mybir
import concourse.tile as tile
from concourse.bass_utils import run_bass_kernel_spmd

P = 128
S = 1024          # sequence = 32*32
C = 512           # channels
NH = 8            # heads
HD = 64           # head dim
CT = C // P       # 4 c-tiles
MT = S // P       # 8 s-tiles
BF = mybir.dt.bfloat16
F32 = mybir.dt.float32
I16 = mybir.dt.int16

# Schraudolph exp: bf16 bits = round(A*s + B), folding the 1/sqrt(64) scale
SCH_A = float((2.0**7) / np.log(2.0) / 8.0)
SCH_B = float(127 * 2**7 - 7.5)

# which (mt, nt) slots per step use DVE Schraudolph instead of ACT exp
DVE_SLOTS = {0: (), 1: (1, 5, 9, 13), 2: (1, 5, 9, 13), 3: (1, 5, 9, 13), 4: ()}

_cache = {}


def build_program():
    nc = bacc.Bacc("TRN2", target_bir_lowering=False, debug=False, num_devices=8)
    # host pre-packed layouts (see _prep)
    x_d = nc.dram_tensor("x", [P, CT * S], BF, kind="ExternalInput").ap()
    wq_d = nc.dram_tensor("wq", [P, CT * 3 * C], BF, kind="ExternalInput").ap()
    wo_d = nc.dram_tensor("wo", [P, CT * C], BF, kind="ExternalInput").ap()
    y_d = nc.dram_tensor("y", [C, S], BF, kind="ExternalOutput").ap()
    wsink_d = nc.dram_tensor("wsink", [1, 8], BF, kind="ExternalOutput").ap()

    with tile.TileContext(nc) as tc:
        with (
            tc.tile_pool(name="const", bufs=1) as cpool,
            tc.tile_pool(name="qk", bufs=1) as qkpool,
            tc.tile_pool(name="vp", bufs=1) as vpool,
            tc.tile_pool(name="pp", bufs=32) as ppool,
            tc.tile_pool(name="ot", bufs=1) as opool,
            tc.tile_pool(name="misc", bufs=4) as mpool,
            tc.tile_pool(name="psq", bufs=3, space="PSUM") as psq_pool,
            tc.tile_pool(name="pso", bufs=2, space="PSUM") as pso_pool,
        ):
            # ---- PE warmup: keep HAM busy while input DMAs land ----
            warm = cpool.tile([P, 512], BF, name="warm", tag="warm")
            nc.scalar.memzero(warm[:])
            warm_ps = psq_pool.tile([P, 1024], F32, name="wps", tag="psq")
            for _ in range(14):
                nc.tensor.matmul(
                    warm_ps[:, 0:512], warm[:, 0:128], warm[:],
                    start=True, stop=True,
                )
            wsink = cpool.tile([1, 8], BF, name="wsink", tag="wsink")
            nc.vector.tensor_copy(wsink[:], warm_ps[0:1, 0:8])

            # ---- load inputs: 7 contiguous DMAs, one queue, need order ----
            # x host layout: [p, nt*2048 + ct*512 + s']  (nt-major)
            x_sb = cpool.tile([P, CT * S], BF, name="xall", tag="xall")
            w_sb = cpool.tile([P, CT * 3 * C], BF, name="wall", tag="wall")
            wo_sb = cpool.tile([P, CT * C], BF, name="woall", tag="woall")
            # w host layout: [q0k0 | q1k1 | v | q2k2 | q3k3], ct-major inside
            nc.sync.dma_start(w_sb[:, 0:1024], wq_d[:, 0:1024])        # q0k0
            nc.sync.dma_start(x_sb[:, 0:2048], x_d[:, 0:2048])         # x nt0
            nc.sync.dma_start(x_sb[:, 2048:4096], x_d[:, 2048:4096])   # x nt1
            nc.sync.dma_start(w_sb[:, 1024:2048], wq_d[:, 1024:2048])  # q1k1
            nc.sync.dma_start(w_sb[:, 2048:4096], wq_d[:, 2048:4096])  # v
            nc.sync.dma_start(w_sb[:, 4096:6144], wq_d[:, 4096:6144])  # q2k2 q3k3
            nc.sync.dma_start(wo_sb[:], wo_d[:, :])
            nc.sync.dma_start(wsink_d[:, :], wsink[:])
            W_PAIR_BASE = (0, 1024, 4096, 5120)

            def xs(ct, lo, hi):
                # x cols [lo:hi) of c-tile ct in nt-major layout (hi-lo <= 512
                # and the range must not straddle the nt boundary at 512)
                nt, off = lo // 512, lo % 512
                base = nt * 2048 + ct * 512 + off
                return x_sb[:, base: base + (hi - lo)]

            # ---- qkT projection: qk_sb[pair] = q-pair, qk_sb[4+pair] = k-pair ----
            qk_sb = [qkpool.tile([P, S], BF, name=f"qk{et}", tag=f"qk{et}")
                     for et in range(8)]
            v_sb = [None] * MT

            def emit_qk_group(pair, kq, nt):
                ps = pso_pool.tile([P, 512], F32, name="qp", tag="pso")
                for ct in range(CT):
                    base = W_PAIR_BASE[pair] + ct * 256 + kq * 128
                    nc.tensor.matmul(
                        ps[:],
                        w_sb[:, base:base + 128],
                        xs(ct, nt * 512, (nt + 1) * 512),
                        start=(ct == 0), stop=(ct == CT - 1),
                    )
                nc.vector.tensor_copy(
                    qk_sb[4 * kq + pair][:, nt * 512:(nt + 1) * 512], ps[:]
                )

            def emit_v_group(mt):
                ps = pso_pool.tile([P, 512], F32, name="vps", tag="pso")
                for ct in range(CT):
                    nc.tensor.matmul(
                        ps[:],
                        xs(ct, mt * P, (mt + 1) * P),
                        w_sb[:, 2048 + ct * 512: 2048 + (ct + 1) * 512],
                        start=(ct == 0), stop=(ct == CT - 1),
                    )
                vt = vpool.tile([P, NH * (HD + 1)], BF, name=f"v{mt}", tag=f"v{mt}")
                nc.gpsimd.memset(vt[:], 1.0)
                dst = vt[:].rearrange("p (h e) -> p h e", e=HD + 1)[:, :, 0:HD]
                nc.vector.tensor_copy(dst, ps[:].rearrange("p (h e) -> p h e", e=HD))
                v_sb[mt] = vt

            # block A: what pair-0 attention needs first
            for pair, kq, nt in ((0, 0, 0), (0, 1, 0), (0, 0, 1), (0, 1, 1)):
                emit_qk_group(pair, kq, nt)
            pending = [("qk", 1, 0, 0), ("qk", 1, 1, 0), ("qk", 1, 0, 1), ("qk", 1, 1, 1)]
            pending += [("v", mt, None, None) for mt in range(MT)]
            pending += [("qk", pr, kq, nt) for pr in (2, 3) for kq in (0, 1)
                        for nt in (0, 1)]
            pend_i = 0

            # ---- attention, software-pipelined: QK/exp(pair p) || PV(pair p-1) ----
            oT_sb = [opool.tile([P, S], BF, name=f"o{ct}", tag=f"o{ct}")
                     for ct in range(CT)]
            y_sb = [opool.tile([P, 512], BF, name=f"ysb{i}", tag=f"ysb{i}")
                    for i in range(8)]
            p_tiles = {}
            DRAIN_ORDER = ((0, 0), (1, 0), (0, 1), (1, 1))

            def emit_qk_slot(step, mt, nt, dve_tiles=()):
                psq = psq_pool.tile([P, 1024], F32, name="psq", tag="psq")
                for hh in range(2):
                    nc.tensor.matmul(
                        psq[:, hh * 512:(hh + 1) * 512],
                        qk_sb[4 + step][hh * HD:(hh + 1) * HD, mt * P:(mt + 1) * P],
                        qk_sb[step][hh * HD:(hh + 1) * HD, nt * 512:(nt + 1) * 512],
                        start=True, stop=True,
                    )
                pt = ppool.tile([P, 1024], BF, name="ptile", tag="ptile")
                if mt * 2 + nt in DVE_SLOTS[step]:
                    nc.vector.tensor_scalar(
                        pt[:].bitcast(I16), psq[:], SCH_A, SCH_B,
                        mybir.AluOpType.mult, mybir.AluOpType.add,
                    )
                else:
                    nc.scalar.activation(
                        pt[:], psq[:], mybir.ActivationFunctionType.Exp,
                        scale=float(1.0 / np.sqrt(HD)),
                    )
                p_tiles[(step, mt, nt)] = pt

            def emit_pv(pp_, mt, idx, hh, nt, pso_t):
                h = 2 * pp_ + hh
                nc.tensor.matmul(
                    pso_t[idx][0:HD + 1, :],
                    v_sb[mt][:, h * (HD + 1):(h + 1) * (HD + 1)],
                    p_tiles[(pp_, mt, nt)][:, hh * 512:(hh + 1) * 512],
                    start=(mt == 0), stop=(mt == MT - 1),
                )

            def normalize(pp_, idx, hh, nt, pso_t, copy_eng="v"):
                h = 2 * pp_ + hh
                ct, half = h // 2, h % 2
                rrow = mpool.tile([1, 512], F32, name="rrow", tag="rrow")
                if copy_eng == "v":
                    nc.vector.tensor_copy(rrow[0:1, :], pso_t[idx][HD:HD + 1, :])
                else:
                    nc.scalar.copy(rrow[0:1, :], pso_t[idx][HD:HD + 1, :])
                rinv = mpool.tile([1, 512], F32, name="rinv", tag="rinv")
                nc.vector.reciprocal_approx_fast(rinv[0:1, :], rrow[0:1, :])
                bc = mpool.tile([HD, 512], F32, name="bc", tag="bc")
                nc.gpsimd.partition_broadcast(bc[:], rinv[0:1, :], channels=HD)
                nc.vector.tensor_mul(
                    oT_sb[ct][half * HD:(half + 1) * HD, nt * 512:(nt + 1) * 512],
                    pso_t[idx][0:HD, :], bc[:],
                )

            def proj_mm(ot, ct, st_, chunk, start, stop):
                nc.tensor.matmul(
                    chunk,
                    wo_sb[:, ct * 512 + ot * P: ct * 512 + (ot + 1) * P],
                    oT_sb[ct][:, st_ * 512:(st_ + 1) * 512],
                    start=start, stop=stop, skip_group_check=True,
                )

            def emit_y_one(ot, st_, chunk, copy_eng, dma_eng):
                dst = y_sb[st_ * 4 + ot][:]
                if copy_eng == "v":
                    nc.vector.tensor_copy(dst, chunk)
                else:
                    nc.scalar.copy(dst, chunk)
                eng = nc.gpsimd if dma_eng == "g" else nc.sync
                eng.dma_start(
                    y_d[ot * P:(ot + 1) * P, st_ * 512:(st_ + 1) * 512], dst)

            # step 0: QK/exp for pair 0 + paced drain of remaining qkv groups
            for mt in range(MT):
                for nt in range(2):
                    emit_qk_slot(0, mt, nt)
                    slot = mt * 2 + nt
                    want = 20 * (slot + 1) // 16
                    while pend_i < min(want, 20):
                        kind, i1, i2, i3 = pending[pend_i]
                        if kind == "qk":
                            emit_qk_group(i1, i2, i3)
                        else:
                            emit_v_group(i1)
                        pend_i += 1

            # steps 1..3: QK slots interleaved with group-major PV of pair-1;
            # each PV group normalizes mid-step, freeing its PSUM bank early
            for step in range(1, 4):
                cur = [None] * 4
                for j in range(16):
                    emit_qk_slot(step, j // 2, j % 2)
                    for k in (2 * j, 2 * j + 1):
                        g, mt = k // MT, k % MT
                        hh, nt = DRAIN_ORDER[g]
                        if mt == 0:
                            cur[g] = pso_pool.tile([P, 512], F32,
                                                   name=f"pso{g}", tag="pso")
                        emit_pv(step - 1, mt, g, hh, nt, cur)
                        if mt == MT - 1:
                            normalize(step - 1, g, hh, nt, cur)

            # step 4: PV pair 3 group-major + staggered normalize + out-proj waves
            pso_t4 = [None] * 4
            wvA = [psq_pool.tile([P, 1024], F32, name=f"wa{i}", tag="psq")
                   for i in range(2)]
            wvA_c = [wvA[0][:, 0:512], wvA[0][:, 512:1024],
                     wvA[1][:, 0:512], wvA[1][:, 512:1024]]
            for g, (hh, nt) in enumerate(DRAIN_ORDER):
                pso_t4[g] = pso_pool.tile([P, 512], F32, name=f"ps4{g}", tag="pso")
                for mt in range(MT):
                    emit_pv(3, mt, g, hh, nt, pso_t4)
                normalize(3, g, hh, nt, pso_t4,
                          copy_eng="s" if g < 3 else "v")
                if g == 1:
                    for ct in (0, 1, 2):
                        for ot in range(CT):
                            proj_mm(ot, ct, 0, wvA_c[ot], ct == 0, False)
                if g == 2:
                    for ot in range(CT):
                        proj_mm(ot, 3, 0, wvA_c[ot], False, True)
                        emit_y_one(ot, 0, wvA_c[ot], "s", "s")
            # wave B: on recycled psq tiles
            wvB = [psq_pool.tile([P, 1024], F32, name=f"wb{i}", tag="psq")
                   for i in range(2)]
            wvB_c = [wvB[0][:, 0:512], wvB[0][:, 512:1024],
                     wvB[1][:, 0:512], wvB[1][:, 512:1024]]
            for ct in (0, 1, 2):
                for ot in range(CT):
                    proj_mm(ot, ct, 1, wvB_c[ot], ct == 0, False)
            for ot in range(CT):
                proj_mm(ot, 3, 1, wvB_c[ot], False, True)
                emit_y_one(ot, 1, wvB_c[ot],
                           "v" if ot < 2 else "s", "s")

    nc.compile()
    return nc


def get_program():
    if "nc" not in _cache:
        _cache["nc"] = build_program()
    return _cache["nc"]


def _prep(x, w_qkv, w_out):
    N = x.shape[0]
    # x: [N, C, H, W] -> [N, 128, nt*2048 + ct*512 + s'] (nt-major)
    xb = (x.reshape(N, CT, P, 2, 512).transpose(0, 2, 3, 1, 4)
          .reshape(N, P, CT * S))
    xb = np.ascontiguousarray(xb).astype(ml_dtypes.bfloat16)
    # w: [3C, C] -> wqT [C, 3C] -> [q0k0 | q1k1 | v | q2k2 | q3k3], ct-major
    wqT = w_qkv.T.astype(np.float32)                    # [C, 3C] cols e
    W = wqT.reshape(CT, P, 3 * C)                       # [ct, p, e]

    def ctmaj(b):                                       # [ct, p, w] -> [p, ct*w]
        return b.transpose(1, 0, 2).reshape(P, -1)

    pairs = [
        np.concatenate([W[:, :, pr * 128:(pr + 1) * 128],
                        W[:, :, 512 + pr * 128:512 + (pr + 1) * 128]], axis=2)
        for pr in range(4)
    ]
    vblk = W[:, :, 1024:1536]
    wq2 = np.concatenate(
        [ctmaj(pairs[0]), ctmaj(pairs[1]), ctmaj(vblk),
         ctmaj(pairs[2]), ctmaj(pairs[3])], axis=1)
    wq2 = np.ascontiguousarray(wq2).astype(ml_dtypes.bfloat16)
    woT = w_out.T.astype(np.float32)                    # [C, C]
    wo2 = woT.reshape(CT, P, C).transpose(1, 0, 2).reshape(P, CT * C)
    wo2 = np.ascontiguousarray(wo2).astype(ml_dtypes.bfloat16)
    return xb, wq2, wo2


def kernel(x, w_qkv, w_out, b_out, _trace=False, _tmpdir=None):
    x = np.asarray(x, dtype=np.float32)
    w_qkv = np.asarray(w_qkv, dtype=np.float32)
    w_out = np.asarray(w_out, dtype=np.float32)
    b_out = np.asarray(b_out, dtype=np.float32)
    N = x.shape[0]

    xb, wq2, wo2 = _prep(x, w_qkv, w_out)
    nc = get_program()
    in_maps = [
        {"x": np.ascontiguousarray(xb[n]), "wq": wq2, "wo": wo2}
        for n in range(N)
    ]
    res = run_bass_kernel_spmd(
        nc, in_maps, core_ids=list(range(N)), trace=_trace, tmpdir=_tmpdir
    )
    y = np.stack([np.asarray(res.results[n]["y"]).astype(np.float32)
                  for n in range(N)])
    y = y.reshape(N, C, 32, 32)
    y = y + b_out[None, :, None, None]
    if _trace:
        return y, res
    return y


# revision 4
# speedup vs baseline: 1.0124x; 1.0013x over previous
"""MHA kernel for TRN2: x[8,512,32,32], 8 heads, S=1024, C=512.

Sharding: data-parallel over batch N=8 -> one batch item per NeuronCore.
Per-core layout (all transpose-free):
  qkT[e,s]  = w.T @ x                     (e on partitions)
  v[s,e]    = x.T @ w_v                   (s on partitions)
  scoresT   = kT_h.T @ qT_h               (k_s on partitions; K=64 row-tiled head
                                           pair runs concurrently on the PE)
  P         = exp(scoresT/8)              (ACT exact; 4 tiles/step on DVE via a
                                           Schraudolph int16 bit-trick)
  oT_aug    = [v_h | 1].T @ P             (M=65; row 64 = softmax denominator r)
  oT        = oT_aug[:64] * (1/r)         (DVE recip + gpsimd broadcast + DVE mul)
  yT[o,s]   = w_outT.T @ oT               (bf16 out; bias added host-side)

Schedule: host pre-packs x/w/wo into [128, *] layouts so input DMA is 7 large
contiguous need-ordered transfers on one HWDGE queue. 14 warmup matmuls bridge
the DMA window so the PE HAM clock is warm when real work starts (kept alive by
the wsink output). Attention is software-pipelined: QK/exp of pair p overlaps
PV of pair p-1; PV runs group-major so each softmax normalize happens mid-step
and frees its PSUM bank early. PSUM: 3x[128,1024] score tiles (loose QK<->exp
coupling) + 2 rotating PV banks. The output projection's two 512-column waves
overlap the final PV drain on recycled PSUM banks.
"""

import numpy as np
import ml_dtypes

import concourse.bacc as bacc
import concourse.mybir as mybir
import concourse.tile as tile
from concourse.bass_utils import run_bass_kernel_spmd

P = 128
S = 1024          # sequence = 32*32
C = 512           # channels
NH = 8            # heads
HD = 64           # head dim
CT = C // P       # 4 c-tiles
MT = S // P       # 8 s-tiles
BF = mybir.dt.bfloat16
F32 = mybir.dt.float32
I16 = mybir.dt.int16

# Schraudolph exp: bf16 bits = round(A*s + B), folding the 1/sqrt(64) scale
SCH_A = float((2.0**7) / np.log(2.0) / 8.0)
SCH_B = float(127 * 2**7 - 7.5)

# which (mt, nt) slots per step use DVE Schraudolph instead of ACT exp
DVE_SLOTS = {0: (), 1: (1, 5, 9, 13), 2: (1, 5, 9, 13), 3: (1, 5, 9, 13), 4: ()}

_cache = {}


def build_program():
    nc = bacc.Bacc("TRN2", target_bir_lowering=False, debug=False, num_devices=8)
    # host pre-packed layouts (see _prep)
    x_d = nc.dram_tensor("x", [P, CT * S], BF, kind="ExternalInput").ap()
    wq_d = nc.dram_tensor("wq", [P, CT * 3 * C], BF, kind="ExternalInput").ap()
    wo_d = nc.dram_tensor("wo", [P, CT * C], BF, kind="ExternalInput").ap()
    y_d = nc.dram_tensor("y", [C, S], BF, kind="ExternalOutput").ap()
    wsink_d = nc.dram_tensor("wsink", [1, 8], BF, kind="ExternalOutput").ap()

    with tile.TileContext(nc) as tc:
        with (
            tc.tile_pool(name="const", bufs=1) as cpool,
            tc.tile_pool(name="qk", bufs=1) as qkpool,
            tc.tile_pool(name="vp", bufs=1) as vpool,
            tc.tile_pool(name="pp", bufs=32) as ppool,
            tc.tile_pool(name="ot", bufs=1) as opool,
            tc.tile_pool(name="misc", bufs=4) as mpool,
            tc.tile_pool(name="psq", bufs=3, space="PSUM") as psq_pool,
            tc.tile_pool(name="pso", bufs=2, space="PSUM") as pso_pool,
        ):
            # ---- PE warmup: keep HAM busy while input DMAs land ----
            warm = cpool.tile([P, 512], BF, name="warm", tag="warm")
            nc.gpsimd.memset(warm[:], 0.0)
            warm_ps = psq_pool.tile([P, 1024], F32, name="wps", tag="psq")
            for _ in range(14):
                nc.tensor.matmul(
                    warm_ps[:, 0:512], warm[:, 0:128], warm[:],
                    start=True, stop=True,
                )
            wsink = cpool.tile([1, 8], BF, name="wsink", tag="wsink")
            nc.vector.tensor_copy(wsink[:], warm_ps[0:1, 0:8])

            # ---- load inputs: 7 contiguous DMAs, one queue, need order ----
            # x host layout: [p, nt*2048 + ct*512 + s']  (nt-major)
            x_sb = cpool.tile([P, CT * S], BF, name="xall", tag="xall")
            w_sb = cpool.tile([P, CT * 3 * C], BF, name="wall", tag="wall")
            wo_sb = cpool.tile([P, CT * C], BF, name="woall", tag="woall")
            # w host layout: [q0k0 | q1k1 | v | q2k2 | q3k3], ct-major inside
            nc.sync.dma_start(w_sb[:, 0:1024], wq_d[:, 0:1024])        # q0k0
            nc.sync.dma_start(x_sb[:, 0:2048], x_d[:, 0:2048])         # x nt0
            nc.sync.dma_start(x_sb[:, 2048:4096], x_d[:, 2048:4096])   # x nt1
            nc.sync.dma_start(w_sb[:, 1024:2048], wq_d[:, 1024:2048])  # q1k1
            nc.sync.dma_start(w_sb[:, 2048:4096], wq_d[:, 2048:4096])  # v
            nc.sync.dma_start(w_sb[:, 4096:6144], wq_d[:, 4096:6144])  # q2k2 q3k3
            nc.sync.dma_start(wo_sb[:], wo_d[:, :])
            nc.sync.dma_start(wsink_d[:, :], wsink[:])
            W_PAIR_BASE = (0, 1024, 4096, 5120)

            def xs(ct, lo, hi):
                # x cols [lo:hi) of c-tile ct in nt-major layout (hi-lo <= 512
                # and the range must not straddle the nt boundary at 512)
                nt, off = lo // 512, lo % 512
                base = nt * 2048 + ct * 512 + off
                return x_sb[:, base: base + (hi - lo)]

            # ---- qkT projection: qk_sb[pair] = q-pair, qk_sb[4+pair] = k-pair ----
            qk_sb = [qkpool.tile([P, S], BF, name=f"qk{et}", tag=f"qk{et}")
                     for et in range(8)]
            v_sb = [None] * MT

            def emit_qk_group(pair, kq, nt):
                ps = pso_pool.tile([P, 512], F32, name="qp", tag="pso")
                for ct in range(CT):
                    base = W_PAIR_BASE[pair] + ct * 256 + kq * 128
                    nc.tensor.matmul(
                        ps[:],
                        w_sb[:, base:base + 128],
                        xs(ct, nt * 512, (nt + 1) * 512),
                        start=(ct == 0), stop=(ct == CT - 1),
                    )
                nc.vector.tensor_copy(
                    qk_sb[4 * kq + pair][:, nt * 512:(nt + 1) * 512], ps[:]
                )

            def emit_v_group(mt):
                ps = pso_pool.tile([P, 512], F32, name="vps", tag="pso")
                for ct in range(CT):
                    nc.tensor.matmul(
                        ps[:],
                        xs(ct, mt * P, (mt + 1) * P),
                        w_sb[:, 2048 + ct * 512: 2048 + (ct + 1) * 512],
                        start=(ct == 0), stop=(ct == CT - 1),
                    )
                vt = vpool.tile([P, NH * (HD + 1)], BF, name=f"v{mt}", tag=f"v{mt}")
                nc.gpsimd.memset(vt[:], 1.0)
                dst = vt[:].rearrange("p (h e) -> p h e", e=HD + 1)[:, :, 0:HD]
                nc.vector.tensor_copy(dst, ps[:].rearrange("p (h e) -> p h e", e=HD))
                v_sb[mt] = vt

            # block A: what pair-0 attention needs first
            for pair, kq, nt in ((0, 0, 0), (0, 1, 0), (0, 0, 1), (0, 1, 1)):
                emit_qk_group(pair, kq, nt)
            pending = [("qk", 1, 0, 0), ("qk", 1, 1, 0), ("qk", 1, 0, 1), ("qk", 1, 1, 1)]
            pending += [("v", mt, None, None) for mt in range(MT)]
            pending += [("qk", pr, kq, nt) for pr in (2, 3) for kq in (0, 1)
                        for nt in (0, 1)]
            pend_i = 0

            # ---- attention, software-pipelined: QK/exp(pair p) || PV(pair p-1) ----
            oT_sb = [opool.tile([P, S], BF, name=f"o{ct}", tag=f"o{ct}")
                     for ct in range(CT)]
            y_sb = [opool.tile([P, 512], BF, name=f"ysb{i}", tag=f"ysb{i}")
                    for i in range(8)]
            p_tiles = {}
            DRAIN_ORDER = ((0, 0), (1, 0), (0, 1), (1, 1))

            def emit_qk_slot(step, mt, nt, dve_tiles=()):
                psq = psq_pool.tile([P, 1024], F32, name="psq", tag="psq")
                for hh in range(2):
                    nc.tensor.matmul(
                        psq[:, hh * 512:(hh + 1) * 512],
                        qk_sb[4 + step][hh * HD:(hh + 1) * HD, mt * P:(mt + 1) * P],
                        qk_sb[step][hh * HD:(hh + 1) * HD, nt * 512:(nt + 1) * 512],
                        start=True, stop=True,
                    )
                pt = ppool.tile([P, 1024], BF, name="ptile", tag="ptile")
                if mt * 2 + nt in DVE_SLOTS[step]:
                    nc.vector.tensor_scalar(
                        pt[:].bitcast(I16), psq[:], SCH_A, SCH_B,
                        mybir.AluOpType.mult, mybir.AluOpType.add,
                    )
                else:
                    nc.scalar.activation(
                        pt[:], psq[:], mybir.ActivationFunctionType.Exp,
                        scale=float(1.0 / np.sqrt(HD)),
                    )
                p_tiles[(step, mt, nt)] = pt

            def emit_pv(pp_, mt, idx, hh, nt, pso_t):
                h = 2 * pp_ + hh
                nc.tensor.matmul(
                    pso_t[idx][0:HD + 1, :],
                    v_sb[mt][:, h * (HD + 1):(h + 1) * (HD + 1)],
                    p_tiles[(pp_, mt, nt)][:, hh * 512:(hh + 1) * 512],
                    start=(mt == 0), stop=(mt == MT - 1),
                )

            def normalize(pp_, idx, hh, nt, pso_t, copy_eng="v"):
                h = 2 * pp_ + hh
                ct, half = h // 2, h % 2
                rrow = mpool.tile([1, 512], F32, name="rrow", tag="rrow")
                if copy_eng == "v":
                    nc.vector.tensor_copy(rrow[0:1, :], pso_t[idx][HD:HD + 1, :])
                else:
                    nc.scalar.copy(rrow[0:1, :], pso_t[idx][HD:HD + 1, :])
                rinv = mpool.tile([1, 512], F32, name="rinv", tag="rinv")
                nc.vector.reciprocal_approx_fast(rinv[0:1, :], rrow[0:1, :])
                bc = mpool.tile([HD, 512], F32, name="bc", tag="bc")
                nc.gpsimd.partition_broadcast(bc[:], rinv[0:1, :], channels=HD)
                nc.vector.tensor_mul(
                    oT_sb[ct][half * HD:(half + 1) * HD, nt * 512:(nt + 1) * 512],
                    pso_t[idx][0:HD, :], bc[:],
                )

            def proj_mm(ot, ct, st_, chunk, start, stop):
                nc.tensor.matmul(
                    chunk,
                    wo_sb[:, ct * 512 + ot * P: ct * 512 + (ot + 1) * P],
                    oT_sb[ct][:, st_ * 512:(st_ + 1) * 512],
                    start=start, stop=stop, skip_group_check=True,
                )

            def emit_y_one(ot, st_, chunk, copy_eng, dma_eng):
                dst = y_sb[st_ * 4 + ot][:]
                if copy_eng == "v":
                    nc.vector.tensor_copy(dst, chunk)
                else:
                    nc.scalar.copy(dst, chunk)
                eng = nc.gpsimd if dma_eng == "g" else nc.sync
                eng.dma_start(
                    y_d[ot * P:(ot + 1) * P, st_ * 512:(st_ + 1) * 512], dst)

            # step 0: QK/exp for pair 0 + paced drain of remaining qkv groups
            for mt in range(MT):
                for nt in range(2):
                    emit_qk_slot(0, mt, nt)
                    slot = mt * 2 + nt
                    want = 20 * (slot + 1) // 16
                    while pend_i < min(want, 20):
                        kind, i1, i2, i3 = pending[pend_i]
                        if kind == "qk":
                            emit_qk_group(i1, i2, i3)
                        else:
                            emit_v_group(i1)
                        pend_i += 1

            # steps 1..3: QK slots interleaved with group-major PV of pair-1;
            # each PV group normalizes mid-step, freeing its PSUM bank early
            for step in range(1, 4):
                cur = [None] * 4
                for j in range(16):
                    emit_qk_slot(step, j // 2, j % 2)
                    for k in (2 * j, 2 * j + 1):
                        g, mt = k // MT, k % MT
                        hh, nt = DRAIN_ORDER[g]
                        if mt == 0:
                            cur[g] = pso_pool.tile([P, 512], F32,
                                                   name=f"pso{g}", tag="pso")
                        emit_pv(step - 1, mt, g, hh, nt, cur)
                        if mt == MT - 1:
                            normalize(step - 1, g, hh, nt, cur)

            # step 4: PV pair 3 group-major + staggered normalize + out-proj waves
            pso_t4 = [None] * 4
            wvA = [psq_pool.tile([P, 1024], F32, name=f"wa{i}", tag="psq")
                   for i in range(2)]
            wvA_c = [wvA[0][:, 0:512], wvA[0][:, 512:1024],
                     wvA[1][:, 0:512], wvA[1][:, 512:1024]]
            for g, (hh, nt) in enumerate(DRAIN_ORDER):
                pso_t4[g] = pso_pool.tile([P, 512], F32, name=f"ps4{g}", tag="pso")
                for mt in range(MT):
                    emit_pv(3, mt, g, hh, nt, pso_t4)
                normalize(3, g, hh, nt, pso_t4,
                          copy_eng="s" if g < 3 else "v")
                if g == 1:
                    for ct in (0, 1, 2):
                        for ot in range(CT):
                            proj_mm(ot, ct, 0, wvA_c[ot], ct == 0, False)
                if g == 2:
                    for ot in range(CT):
                        proj_mm(ot, 3, 0, wvA_c[ot], False, True)
                        emit_y_one(ot, 0, wvA_c[ot], "s", "s")
            # wave B: on recycled psq tiles
            wvB = [psq_pool.tile([P, 1024], F32, name=f"wb{i}", tag="psq")
                   for i in range(2)]
            wvB_c = [wvB[0][:, 0:512], wvB[0][:, 512:1024],
                     wvB[1][:, 0:512], wvB[1][:, 512:1024]]
            for ct in (0, 1, 2):
                for ot in range(CT):
                    proj_mm(ot, ct, 1, wvB_c[ot], ct == 0, False)
            for ot in range(CT):
                proj_mm(ot, 3, 1, wvB_c[ot], False, True)
                emit_y_one(ot, 1, wvB_c[ot],
                           "v" if ot < 2 else "s", "s")

    nc.compile()
    return nc


def get_program():
    if "nc" not in _cache:
        _cache["nc"] = build_program()
    return _cache["nc"]


def _prep(x, w_qkv, w_out):
    N = x.shape[0]
    # x: [N, C, H, W] -> [N, 128, nt*2048 + ct*512 + s'] (nt-major)
    xb = (x.reshape(N, CT, P, 2, 512).transpose(0, 2, 3, 1, 4)
          .reshape(N, P, CT * S))
    xb = np.ascontiguousarray(xb).astype(ml_dtypes.bfloat16)
    # w: [3C, C] -> wqT [C, 3C] -> [q0k0 | q1k1 | v | q2k2 | q3k3], ct-major
    wqT = w_qkv.T.astype(np.float32)                    # [C, 3C] cols e
    W = wqT.reshape(CT, P, 3 * C)                       # [ct, p, e]

    def ctmaj(b):                                       # [ct, p, w] -> [p, ct*w]
        return b.transpose(1, 0, 2).reshape(P, -1)

    pairs = [
        np.concatenate([W[:, :, pr * 128:(pr + 1) * 128],
                        W[:, :, 512 + pr * 128:512 + (pr + 1) * 128]], axis=2)
        for pr in range(4)
    ]
    vblk = W[:, :, 1024:1536]
    wq2 = np.concatenate(
        [ctmaj(pairs[0]), ctmaj(pairs[1]), ctmaj(vblk),
         ctmaj(pairs[2]), ctmaj(pairs[3])], axis=1)
    wq2 = np.ascontiguousarray(wq2).astype(ml_dtypes.bfloat16)
    woT = w_out.T.astype(np.float32)                    # [C, C]
    wo2 = woT.reshape(CT, P, C).transpose(1, 0, 2).reshape(P, CT * C)
    wo2 = np.ascontiguousarray(wo2).astype(ml_dtypes.bfloat16)
    return xb, wq2, wo2


def kernel(x, w_qkv, w_out, b_out, _trace=False, _tmpdir=None):
    x = np.asarray(x, dtype=np.float32)
    w_qkv = np.asarray(w_qkv, dtype=np.float32)
    w_out = np.asarray(w_out, dtype=np.float32)
    b_out = np.asarray(b_out, dtype=np.float32)
    N = x.shape[0]

    xb, wq2, wo2 = _prep(x, w_qkv, w_out)
    nc = get_program()
    in_maps = [
        {"x": np.ascontiguousarray(xb[n]), "wq": wq2, "wo": wo2}
        for n in range(N)
    ]
    res = run_bass_kernel_spmd(
        nc, in_maps, core_ids=list(range(N)), trace=_trace, tmpdir=_tmpdir
    )
    y = np.stack([np.asarray(res.results[n]["y"]).astype(np.float32)
                  for n in range(N)])
    y = y.reshape(N, C, 32, 32)
    y = y + b_out[None, :, None, None]
    if _trace:
        return y, res
    return y


# revision 5
# speedup vs baseline: 1.0148x; 1.0024x over previous
"""MHA kernel for TRN2: x[8,512,32,32], 8 heads, S=1024, C=512.

Sharding: data-parallel over batch N=8 -> one batch item per NeuronCore.
Per-core layout (all transpose-free):
  qkT[e,s]  = w.T @ x                     (e on partitions)
  v[s,e]    = x.T @ w_v                   (s on partitions)
  scoresT   = kT_h.T @ qT_h               (k_s on partitions; K=64 row-tiled head
                                           pair runs concurrently on the PE)
  P         = exp(scoresT/8)              (ACT exact; 4 tiles/step on DVE via a
                                           Schraudolph int16 bit-trick)
  oT_aug    = [v_h | 1].T @ P             (M=65; row 64 = softmax denominator r)
  oT        = oT_aug[:64] * (1/r)         (DVE recip + gpsimd broadcast + DVE mul)
  yT[o,s]   = w_outT.T @ oT               (bf16 out; bias added host-side)

Schedule: host pre-packs x/w/wo into [128, *] layouts so input DMA is 7 large
contiguous need-ordered transfers on one HWDGE queue. 14 warmup matmuls bridge
the DMA window so the PE HAM clock is warm when real work starts (kept alive by
the wsink output). Attention is software-pipelined: QK/exp of pair p overlaps
PV of pair p-1; PV runs group-major so each softmax normalize happens mid-step
and frees its PSUM bank early. PSUM: 3x[128,1024] score tiles (loose QK<->exp
coupling) + 2 rotating PV banks. The output projection's two 512-column waves
overlap the final PV drain on recycled PSUM banks.
"""

import numpy as np
import ml_dtypes

import concourse.bacc as bacc
import concourse.mybir as mybir
import concourse.tile as tile
from concourse.bass_utils import run_bass_kernel_spmd

P = 128
S = 1024          # sequence = 32*32
C = 512           # channels
NH = 8            # heads
HD = 64           # head dim
CT = C // P       # 4 c-tiles
MT = S // P       # 8 s-tiles
BF = mybir.dt.bfloat16
F32 = mybir.dt.float32
I16 = mybir.dt.int16

# Schraudolph exp: bf16 bits = round(A*s + B), folding the 1/sqrt(64) scale
SCH_A = float((2.0**7) / np.log(2.0) / 8.0)
SCH_B = float(127 * 2**7 - 7.5)

# which (mt, nt) slots per step use DVE Schraudolph instead of ACT exp
DVE_SLOTS = {0: (), 1: (1, 5, 9, 13), 2: (1, 5, 9, 13), 3: (1, 5, 9, 13), 4: ()}

_cache = {}


def build_program():
    nc = bacc.Bacc("TRN2", target_bir_lowering=False, debug=False, num_devices=8)
    # host pre-packed layouts (see _prep)
    x_d = nc.dram_tensor("x", [P, CT * S], BF, kind="ExternalInput").ap()
    wq_d = nc.dram_tensor("wq", [P, CT * 3 * C], BF, kind="ExternalInput").ap()
    wo_d = nc.dram_tensor("wo", [P, CT * C], BF, kind="ExternalInput").ap()
    y_d = nc.dram_tensor("y", [C, S], BF, kind="ExternalOutput").ap()
    wsink_d = nc.dram_tensor("wsink", [1, 8], BF, kind="ExternalOutput").ap()

    with tile.TileContext(nc) as tc:
        with (
            tc.tile_pool(name="const", bufs=1) as cpool,
            tc.tile_pool(name="qk", bufs=1) as qkpool,
            tc.tile_pool(name="vp", bufs=1) as vpool,
            tc.tile_pool(name="pp", bufs=32) as ppool,
            tc.tile_pool(name="ot", bufs=1) as opool,
            tc.tile_pool(name="misc", bufs=4) as mpool,
            tc.tile_pool(name="psq", bufs=3, space="PSUM") as psq_pool,
            tc.tile_pool(name="pso", bufs=2, space="PSUM") as pso_pool,
        ):
            # ---- PE warmup: keep HAM busy while input DMAs land ----
            warm = cpool.tile([P, 512], BF, name="warm", tag="warm")
            nc.gpsimd.memset(warm[:], 0.0)
            warm_ps = psq_pool.tile([P, 1024], F32, name="wps", tag="psq")
            for _ in range(14):
                nc.tensor.matmul(
                    warm_ps[:, 0:512], warm[:, 0:128], warm[:],
                    start=True, stop=True,
                )
            wsink = cpool.tile([1, 8], BF, name="wsink", tag="wsink")
            nc.vector.tensor_copy(wsink[:], warm_ps[0:1, 0:8])

            # ---- load inputs: 7 contiguous DMAs, one queue, need order ----
            # x host layout: [p, nt*2048 + ct*512 + s']  (nt-major)
            x_sb = cpool.tile([P, CT * S], BF, name="xall", tag="xall")
            w_sb = cpool.tile([P, CT * 3 * C], BF, name="wall", tag="wall")
            wo_sb = cpool.tile([P, CT * C], BF, name="woall", tag="woall")
            # w host layout: [q0k0 | q1k1 | v | q2k2 | q3k3], ct-major inside
            nc.sync.dma_start(w_sb[:, 0:1024], wq_d[:, 0:1024])        # q0k0
            nc.sync.dma_start(x_sb[:, 0:2048], x_d[:, 0:2048])         # x nt0
            nc.sync.dma_start(x_sb[:, 2048:4096], x_d[:, 2048:4096])   # x nt1
            nc.sync.dma_start(w_sb[:, 1024:2048], wq_d[:, 1024:2048])  # q1k1
            nc.sync.dma_start(w_sb[:, 2048:4096], wq_d[:, 2048:4096])  # v
            nc.sync.dma_start(w_sb[:, 4096:6144], wq_d[:, 4096:6144])  # q2k2 q3k3
            nc.sync.dma_start(wo_sb[:], wo_d[:, :])
            nc.sync.dma_start(wsink_d[:, :], wsink[:])
            W_PAIR_BASE = (0, 1024, 4096, 5120)

            def xs(ct, lo, hi):
                # x cols [lo:hi) of c-tile ct in nt-major layout (hi-lo <= 512
                # and the range must not straddle the nt boundary at 512)
                nt, off = lo // 512, lo % 512
                base = nt * 2048 + ct * 512 + off
                return x_sb[:, base: base + (hi - lo)]

            # ---- qkT projection: qk_sb[pair] = q-pair, qk_sb[4+pair] = k-pair ----
            qk_sb = [qkpool.tile([P, S], BF, name=f"qk{et}", tag=f"qk{et}")
                     for et in range(8)]
            v_sb = [None] * MT

            def emit_qk_group(pair, kq, nt, pool=None):
                if pool is None:
                    ps = pso_pool.tile([P, 512], F32, name="qp", tag="pso")
                else:
                    ps = pool.tile([P, 1024], F32, name="qp2", tag="psq")[:, 0:512]
                for ct in range(CT):
                    base = W_PAIR_BASE[pair] + ct * 256 + kq * 128
                    nc.tensor.matmul(
                        ps[:],
                        w_sb[:, base:base + 128],
                        xs(ct, nt * 512, (nt + 1) * 512),
                        start=(ct == 0), stop=(ct == CT - 1),
                    )
                nc.vector.tensor_copy(
                    qk_sb[4 * kq + pair][:, nt * 512:(nt + 1) * 512], ps[:]
                )

            def emit_v_group(mt):
                ps = pso_pool.tile([P, 512], F32, name="vps", tag="pso")
                for ct in range(CT):
                    nc.tensor.matmul(
                        ps[:],
                        xs(ct, mt * P, (mt + 1) * P),
                        w_sb[:, 2048 + ct * 512: 2048 + (ct + 1) * 512],
                        start=(ct == 0), stop=(ct == CT - 1),
                    )
                vt = vpool.tile([P, NH * (HD + 1)], BF, name=f"v{mt}", tag=f"v{mt}")
                nc.gpsimd.memset(vt[:], 1.0)
                dst = vt[:].rearrange("p (h e) -> p h e", e=HD + 1)[:, :, 0:HD]
                nc.vector.tensor_copy(dst, ps[:].rearrange("p (h e) -> p h e", e=HD))
                v_sb[mt] = vt

            # block A: what pair-0 attention needs first
            for pair, kq, nt in ((0, 0, 0), (0, 1, 0), (0, 0, 1), (0, 1, 1)):
                emit_qk_group(pair, kq, nt)
            pending = [("qk", 1, 0, 0), ("qk", 1, 1, 0), ("qk", 1, 0, 1), ("qk", 1, 1, 1)]
            pending += [("v", mt, None, None) for mt in range(MT)]
            pend_i = 0

            # ---- attention, software-pipelined: QK/exp(pair p) || PV(pair p-1) ----
            oT_sb = [opool.tile([P, S], BF, name=f"o{ct}", tag=f"o{ct}")
                     for ct in range(CT)]
            y_sb = [opool.tile([P, 512], BF, name=f"ysb{i}", tag=f"ysb{i}")
                    for i in range(8)]
            p_tiles = {}
            DRAIN_ORDER = ((0, 0), (1, 0), (0, 1), (1, 1))

            def emit_qk_slot(step, mt, nt, dve_tiles=()):
                psq = psq_pool.tile([P, 1024], F32, name="psq", tag="psq")
                for hh in range(2):
                    nc.tensor.matmul(
                        psq[:, hh * 512:(hh + 1) * 512],
                        qk_sb[4 + step][hh * HD:(hh + 1) * HD, mt * P:(mt + 1) * P],
                        qk_sb[step][hh * HD:(hh + 1) * HD, nt * 512:(nt + 1) * 512],
                        start=True, stop=True,
                    )
                pt = ppool.tile([P, 1024], BF, name="ptile", tag="ptile")
                if mt * 2 + nt in DVE_SLOTS[step]:
                    nc.vector.tensor_scalar(
                        pt[:].bitcast(I16), psq[:], SCH_A, SCH_B,
                        mybir.AluOpType.mult, mybir.AluOpType.add,
                    )
                else:
                    nc.scalar.activation(
                        pt[:], psq[:], mybir.ActivationFunctionType.Exp,
                        scale=float(1.0 / np.sqrt(HD)),
                    )
                p_tiles[(step, mt, nt)] = pt

            def emit_pv(pp_, mt, idx, hh, nt, pso_t):
                h = 2 * pp_ + hh
                nc.tensor.matmul(
                    pso_t[idx][0:HD + 1, :],
                    v_sb[mt][:, h * (HD + 1):(h + 1) * (HD + 1)],
                    p_tiles[(pp_, mt, nt)][:, hh * 512:(hh + 1) * 512],
                    start=(mt == 0), stop=(mt == MT - 1),
                )

            def normalize(pp_, idx, hh, nt, pso_t, copy_eng="v"):
                h = 2 * pp_ + hh
                ct, half = h // 2, h % 2
                rrow = mpool.tile([1, 512], F32, name="rrow", tag="rrow")
                if copy_eng == "v":
                    nc.vector.tensor_copy(rrow[0:1, :], pso_t[idx][HD:HD + 1, :])
                else:
                    nc.scalar.copy(rrow[0:1, :], pso_t[idx][HD:HD + 1, :])
                rinv = mpool.tile([1, 512], F32, name="rinv", tag="rinv")
                nc.vector.reciprocal_approx_fast(rinv[0:1, :], rrow[0:1, :])
                bc = mpool.tile([HD, 512], F32, name="bc", tag="bc")
                nc.gpsimd.partition_broadcast(bc[:], rinv[0:1, :], channels=HD)
                nc.vector.tensor_mul(
                    oT_sb[ct][half * HD:(half + 1) * HD, nt * 512:(nt + 1) * 512],
                    pso_t[idx][0:HD, :], bc[:],
                )

            def proj_mm(ot, ct, st_, chunk, start, stop):
                nc.tensor.matmul(
                    chunk,
                    wo_sb[:, ct * 512 + ot * P: ct * 512 + (ot + 1) * P],
                    oT_sb[ct][:, st_ * 512:(st_ + 1) * 512],
                    start=start, stop=stop, skip_group_check=True,
                )

            def emit_y_one(ot, st_, chunk, copy_eng, dma_eng):
                dst = y_sb[st_ * 4 + ot][:]
                if copy_eng == "v":
                    nc.vector.tensor_copy(dst, chunk)
                else:
                    nc.scalar.copy(dst, chunk)
                eng = nc.gpsimd if dma_eng == "g" else nc.sync
                eng.dma_start(
                    y_d[ot * P:(ot + 1) * P, st_ * 512:(st_ + 1) * 512], dst)

            # step 0: QK/exp for pair 0 + paced drain of remaining qkv groups
            for mt in range(MT):
                for nt in range(2):
                    emit_qk_slot(0, mt, nt)
                    slot = mt * 2 + nt
                    want = 12 * (slot + 1) // 16
                    while pend_i < min(want, 12):
                        kind, i1, i2, i3 = pending[pend_i]
                        if kind == "qk":
                            emit_qk_group(i1, i2, i3)
                        else:
                            emit_v_group(i1)
                        pend_i += 1

            # steps 1..3: QK slots interleaved with group-major PV of pair-1;
            # each PV group normalizes mid-step, freeing its PSUM bank early
            for step in range(1, 4):
                cur = [None] * 4
                for j in range(16):
                    emit_qk_slot(step, j // 2, j % 2)
                    if step <= 2 and j in (2, 6, 10, 14):
                        di = (2, 6, 10, 14).index(j)
                        emit_qk_group(step + 1, di // 2, di % 2, pool=psq_pool)
                    for k in (2 * j, 2 * j + 1):
                        g, mt = k // MT, k % MT
                        hh, nt = DRAIN_ORDER[g]
                        if mt == 0:
                            cur[g] = pso_pool.tile([P, 512], F32,
                                                   name=f"pso{g}", tag="pso")
                        emit_pv(step - 1, mt, g, hh, nt, cur)
                        if mt == MT - 1:
                            normalize(step - 1, g, hh, nt, cur)

            # step 4: PV pair 3 group-major + staggered normalize + out-proj waves
            pso_t4 = [None] * 4
            wvA = [psq_pool.tile([P, 1024], F32, name=f"wa{i}", tag="psq")
                   for i in range(2)]
            wvA_c = [wvA[0][:, 0:512], wvA[0][:, 512:1024],
                     wvA[1][:, 0:512], wvA[1][:, 512:1024]]
            for g, (hh, nt) in enumerate(DRAIN_ORDER):
                pso_t4[g] = pso_pool.tile([P, 512], F32, name=f"ps4{g}", tag="pso")
                for mt in range(MT):
                    emit_pv(3, mt, g, hh, nt, pso_t4)
                normalize(3, g, hh, nt, pso_t4,
                          copy_eng="s" if g < 3 else "v")
                if g == 1:
                    for ct in (0, 1, 2):
                        for ot in range(CT):
                            proj_mm(ot, ct, 0, wvA_c[ot], ct == 0, False)
                if g == 2:
                    for ot in range(CT):
                        proj_mm(ot, 3, 0, wvA_c[ot], False, True)
                        emit_y_one(ot, 0, wvA_c[ot], "s", "s")
            # wave B: on recycled psq tiles
            wvB = [psq_pool.tile([P, 1024], F32, name=f"wb{i}", tag="psq")
                   for i in range(2)]
            wvB_c = [wvB[0][:, 0:512], wvB[0][:, 512:1024],
                     wvB[1][:, 0:512], wvB[1][:, 512:1024]]
            for ct in (0, 1, 2):
                for ot in range(CT):
                    proj_mm(ot, ct, 1, wvB_c[ot], ct == 0, False)
            for ot in range(CT):
                proj_mm(ot, 3, 1, wvB_c[ot], False, True)
                emit_y_one(ot, 1, wvB_c[ot],
                           "v" if ot < 2 else "s", "s")

    nc.compile()
    return nc


def get_program():
    if "nc" not in _cache:
        _cache["nc"] = build_program()
    return _cache["nc"]


def _prep(x, w_qkv, w_out):
    N = x.shape[0]
    # x: [N, C, H, W] -> [N, 128, nt*2048 + ct*512 + s'] (nt-major)
    xb = (x.reshape(N, CT, P, 2, 512).transpose(0, 2, 3, 1, 4)
          .reshape(N, P, CT * S))
    xb = np.ascontiguousarray(xb).astype(ml_dtypes.bfloat16)
    # w: [3C, C] -> wqT [C, 3C] -> [q0k0 | q1k1 | v | q2k2 | q3k3], ct-major
    wqT = w_qkv.T.astype(np.float32)                    # [C, 3C] cols e
    W = wqT.reshape(CT, P, 3 * C)                       # [ct, p, e]

    def ctmaj(b):                                       # [ct, p, w] -> [p, ct*w]
        return b.transpose(1, 0, 2).reshape(P, -1)

    pairs = [
        np.concatenate([W[:, :, pr * 128:(pr + 1) * 128],
                        W[:, :, 512 + pr * 128:512 + (pr + 1) * 128]], axis=2)
        for pr in range(4)
    ]
    vblk = W[:, :, 1024:1536]
    wq2 = np.concatenate(
        [ctmaj(pairs[0]), ctmaj(pairs[1]), ctmaj(vblk),
         ctmaj(pairs[2]), ctmaj(pairs[3])], axis=1)
    wq2 = np.ascontiguousarray(wq2).astype(ml_dtypes.bfloat16)
    woT = w_out.T.astype(np.float32)                    # [C, C]
    wo2 = woT.reshape(CT, P, C).transpose(1, 0, 2).reshape(P, CT * C)
    wo2 = np.ascontiguousarray(wo2).astype(ml_dtypes.bfloat16)
    return xb, wq2, wo2


def kernel(x, w_qkv, w_out, b_out, _trace=False, _tmpdir=None):
    x = np.asarray(x, dtype=np.float32)
    w_qkv = np.asarray(w_qkv, dtype=np.float32)
    w_out = np.asarray(w_out, dtype=np.float32)
    b_out = np.asarray(b_out, dtype=np.float32)
    N = x.shape[0]

    xb, wq2, wo2 = _prep(x, w_qkv, w_out)
    nc = get_program()
    in_maps = [
        {"x": np.ascontiguousarray(xb[n]), "wq": wq2, "wo": wo2}
        for n in range(N)
    ]
    res = run_bass_kernel_spmd(
        nc, in_maps, core_ids=list(range(N)), trace=_trace, tmpdir=_tmpdir
    )
    y = np.stack([np.asarray(res.results[n]["y"]).astype(np.float32)
                  for n in range(N)])
    y = y.reshape(N, C, 32, 32)
    y = y + b_out[None, :, None, None]
    if _trace:
        return y, res
    return y
